# revision 14
# baseline (speedup 1.0000x reference)
"""Trainium2 Bass kernel for the DPLSTM language model problem.

Model: emb = emb_W[x]; LSTM over T steps; logits = out @ emb_W.T + dec_b;
return (log_softmax(logits), (h_T, c_T)).

Sharding (8 cores):
  - embedding gather + layout prep on host (sharding/layout only, no FLOPs)
  - x_proj matmul, LSTM recurrence: replicated on all cores (latency-bound,
    sharding would add per-step collectives that cost more than they save)
  - tied decoder matmul + log_softmax: sharded over vocab; one AllReduce of
    per-row sum(exp(logits)) glues the softmax together.

Self-contained: hardcodes all shapes; no sibling imports.
"""

import os

os.environ.setdefault("MYCRO_LOCAL_CACHE", "1")

import numpy as np
import ml_dtypes

BF16 = ml_dtypes.bfloat16

B = 8          # batch (hard requirement of this kernel)
H = 1024       # hidden
KT = H // 128  # K tiles (8)
NCORES = 8
NEG_PAD = -10000.0  # logit value for padded vocab columns (exp -> 0)

_BUILD_CACHE: dict = {}


# ---------------------------------------------------------------------------
# module builder
# ---------------------------------------------------------------------------

def build_module(T: int, VA: int):
    """Build the Bass module. T = seq len (mult of 16), VA = padded per-core
    vocab shard (mult of 512)."""
    import concourse.bass as bass
    import concourse.bacc as bacc
    import concourse.mybir as mybir
    from concourse import tile

    dt = mybir.dt
    AF = mybir.ActivationFunctionType
    ALU = mybir.AluOpType

    T8 = 8 * T            # rows of the [T*B, ...] matrices
    NTB = T8 // 128       # tb tiles of 128 rows
    NVT = VA // 512       # vocab tiles per core
    G = 4 * H             # 4096 gate columns (order i,f,o,g after host perm)

    nc = bacc.Bacc("TRN2", target_bir_lowering=False, debug=False,
                   num_devices=NCORES)

    # ---- I/O -------------------------------------------------------------
    f32, bf16 = dt.float32, dt.bfloat16
    embT_d = nc.dram_tensor("embT", [H, T8], bf16, kind="ExternalInput").ap()
    wih_d = nc.dram_tensor("wihT", [H, G], bf16, kind="ExternalInput").ap()
    wihb_d = nc.dram_tensor("wihb", [1, G], bf16, kind="ExternalInput").ap()
    whh_d = nc.dram_tensor("whhT", [H, G], bf16, kind="ExternalInput").ap()
    ewt_d = nc.dram_tensor("embWT", [H, VA], bf16, kind="ExternalInput").ap()
    decb_d = nc.dram_tensor("decb", [1, VA], bf16, kind="ExternalInput").ap()
    i8_d = nc.dram_tensor("ident8", [8, 8], bf16, kind="ExternalInput").ap()
    ones_d = nc.dram_tensor("ones1", [1, 128], bf16, kind="ExternalInput").ap()

    out_lp = nc.dram_tensor("out_lp", [T8, VA], f32, kind="ExternalOutput").ap()
    out_h = nc.dram_tensor("out_h", [B, H], f32, kind="ExternalOutput").ap()
    out_c = nc.dram_tensor("out_c", [B, H], f32, kind="ExternalOutput").ap()

    with tile.TileContext(nc) as tc:
        with (
            tc.tile_pool(name="dram", bufs=1, space="DRAM") as dpool,
            tc.tile_pool(name="persist", bufs=1) as pp,
        ):
            xp_tiles = [dpool.tile([128, G], bf16, name=f"xpd{tb}")
                        for tb in range(NTB)]
            lg_tiles = [[dpool.tile([128, 512], bf16, name=f"lgd{tb}_{vt}")
                         for vt in range(NVT)] for tb in range(NTB)]
            ar_in = dpool.tile([128, NTB], f32)
            ar_out = dpool.tile([128, NTB], f32, addr_space="Shared")

            # persistent SBUF state; w_sb holds w_ihT during x_proj, then
            # is overwritten with w_hhT for the recurrence.
            w_sb = pp.tile([128, KT * G], bf16)         # 64KB/part
            hsT_sb = pp.tile([128, KT * T8], bf16)      # 32KB/part (full run)
            c_sb = pp.tile([B, H], f32)
            h_sb = pp.tile([B, H], bf16)
            i8_sb = pp.tile([8, 8], bf16)
            ones_sb = pp.tile([1, 128], bf16)
            decb_sb = pp.tile([1, VA], bf16)
            sums_sb = pp.tile([128, NTB], f32)
            nls_sb = pp.tile([128, NTB], f32)

            nc.sync.dma_start(out=i8_sb[:, :], in_=i8_d[:, :])
            nc.sync.dma_start(out=ones_sb[:, :], in_=ones_d[:, :])
            nc.sync.dma_start(out=decb_sb[:, :], in_=decb_d[:, :])
            nc.sync.dma_start(
                out=w_sb[:, :].rearrange("p (k g) -> p k g", k=KT),
                in_=wih_d.rearrange("(k p) g -> p k g", p=128),
            )
            nc.vector.memset(c_sb[:, :], 0.0)

            # ---- phase 1: x_proj = emb @ w_ih.T + bias -> DRAM (bf16) ----
            with (
                tc.tile_pool(name="xp_w", bufs=1) as xwp,
                tc.tile_pool(name="xp_sb", bufs=2) as xsp,
                tc.tile_pool(name="xp_ps", bufs=3, space="PSUM") as xpp,
            ):
                embT_sb = xwp.tile([128, KT * T8], bf16)
                wihb_sb = xwp.tile([1, G], bf16)
                nc.sync.dma_start(
                    out=embT_sb[:, :].rearrange("p (k t) -> p k t", k=KT),
                    in_=embT_d.rearrange("(k p) t -> p k t", p=128),
                )
                nc.sync.dma_start(out=wihb_sb[:, :], in_=wihb_d[:, :])

                for tb in range(NTB):
                    xb = xsp.tile([128, G], bf16, tag="xpb")
                    for gs in range(G // 512):
                        ps = xpp.tile([128, 512], f32, tag="xps")
                        for k in range(KT):
                            nc.tensor.matmul(
                                ps[:, :],
                                lhsT=embT_sb[:, k * T8 + 128 * tb:
                                             k * T8 + 128 * tb + 128],
                                rhs=w_sb[:, k * G + 512 * gs:
                                         k * G + 512 * gs + 512],
                                start=(k == 0), stop=False,
                            )
                        nc.tensor.matmul(
                            ps[:, :], lhsT=ones_sb[:, 0:128],
                            rhs=wihb_sb[:, 512 * gs: 512 * gs + 512],
                            start=False, stop=True,
                        )
                        nc.vector.tensor_copy(
                            xb[:, 512 * gs: 512 * gs + 512], ps[:, :])
                    nc.sync.dma_start(out=xp_tiles[tb][:, :], in_=xb[:, :])

            # swap in w_hhT (reuses the w_ihT SBUF region)
            nc.sync.dma_start(
                out=w_sb[:, :].rearrange("p (k g) -> p k g", k=KT),
                in_=whh_d.rearrange("(k p) g -> p k g", p=128),
            )

            # ---- phase 2: LSTM recurrence --------------------------------
            with (
                tc.tile_pool(name="rec_xp", bufs=3) as rxp,
                tc.tile_pool(name="rec_ew", bufs=2) as rew,
                tc.tile_pool(name="rec_gps", bufs=2, space="PSUM") as rgp,
                tc.tile_pool(name="rec_hps", bufs=2, space="PSUM") as rhp,
            ):
                for t in range(T):
                    xps = rxp.tile([8, G], bf16, tag="xps")
                    r0 = 8 * (t % 16)
                    nc.sync.dma_start(out=xps[:, :],
                                      in_=xp_tiles[t // 16][r0: r0 + 8, :])
                    hp = rhp.tile([128, 64], bf16, tag="hp")
                    for m in range(4):  # H chunks of 256
                        gp = rgp.tile([8, 1024], f32, tag="gp")
                        for gam in range(4):  # gate type i,f,o,g
                            gc = gam * H + 256 * m
                            if t > 0:
                                for k in range(KT):
                                    nc.tensor.matmul(
                                        gp[:, 256 * gam: 256 * gam + 256],
                                        lhsT=hsT_sb[:, k * T8 + 8 * (t - 1):
                                                    k * T8 + 8 * (t - 1) + 8],
                                        rhs=w_sb[:, k * G + gc:
                                                 k * G + gc + 256],
                                        start=(k == 0), stop=False,
                                    )
                            nc.tensor.matmul(
                                gp[:, 256 * gam: 256 * gam + 256],
                                lhsT=i8_sb[:, :],
                                rhs=xps[:, gc: gc + 256],
                                start=(t == 0), stop=True,
                            )
                        # elementwise for chunk m
                        cm = slice(256 * m, 256 * m + 256)
                        sig = rew.tile([8, 768], f32, tag="sig")
                        tg = rew.tile([8, 256], f32, tag="tg")
                        nc.scalar.activation(sig[:, :], gp[:, 0:768], AF.Sigmoid)
                        nc.scalar.activation(tg[:, :], gp[:, 768:1024], AF.Tanh)
                        x1 = rew.tile([8, 256], f32, tag="x1")
                        x2 = rew.tile([8, 256], f32, tag="x2")
                        nc.vector.tensor_tensor(x1[:, :], sig[:, 0:256],
                                                tg[:, :], ALU.mult)
                        nc.vector.tensor_tensor(x2[:, :], sig[:, 256:512],
                                                c_sb[:, cm], ALU.mult)
                        nc.vector.tensor_tensor(c_sb[:, cm], x1[:, :],
                                                x2[:, :], ALU.add)
                        tcn = rew.tile([8, 256], f32, tag="tcn")
                        nc.scalar.activation(tcn[:, :], c_sb[:, cm], AF.Tanh)
                        nc.vector.tensor_tensor(h_sb[:, cm], sig[:, 512:768],
                                                tcn[:, :], ALU.mult)
                        # transpose h chunk -> hp cols [16m:16m+16]
                        for half in range(2):
                            nc.tensor.matmul(
                                hp[:, 16 * m + 8 * half: 16 * m + 8 * half + 8],
                                lhsT=h_sb[:, 256 * m + 128 * half:
                                          256 * m + 128 * half + 128],
                                rhs=i8_sb[:, :],
                                is_transpose=True, start=True, stop=True,
                                skip_group_check=True,
                            )
                    # hp [128, 64] (col k*8+b) -> hsT_sb cols {k*T8 + 8t + b}
                    nc.scalar.activation(
                        hsT_sb[:, :].rearrange(
                            "p (k r) -> p k r", k=KT)[:, :, 8 * t: 8 * t + 8],
                        hp[:, :].rearrange("p (k r) -> p k r", k=KT),
                        AF.Copy,
                    )

            # final h/c outputs
            hf = pp.tile([B, H], f32)
            nc.scalar.activation(hf[:, :], h_sb[:, :], AF.Copy)
            nc.sync.dma_start(out=out_h[:, :], in_=hf[:, :])
            nc.sync.dma_start(out=out_c[:, :], in_=c_sb[:, :])

            # ---- phase 3: decoder + exp-sum ------------------------------
            with (
                tc.tile_pool(name="dec_w", bufs=2) as dwp,
                tc.tile_pool(name="dec_sb", bufs=3) as dsp,
                tc.tile_pool(name="dec_ps", bufs=2, space="PSUM") as dpp,
            ):
                for vt in range(NVT):
                    wt = dwp.tile([128, KT * 512], bf16, tag="wt")
                    nc.sync.dma_start(
                        out=wt[:, :].rearrange("p (k v) -> p k v", k=KT),
                        in_=ewt_d.rearrange("(k p) v -> p k v", p=128)
                        [:, :, 512 * vt: 512 * vt + 512],
                    )
                    for tb in range(NTB):
                        lg = dpp.tile([128, 512], f32, tag="lg")
                        for k in range(KT):
                            nc.tensor.matmul(
                                lg[:, :],
                                lhsT=hsT_sb[:, k * T8 + 128 * tb:
                                            k * T8 + 128 * tb + 128],
                                rhs=wt[:, 512 * k: 512 * k + 512],
                                start=(k == 0), stop=False,
                            )
                        nc.tensor.matmul(
                            lg[:, :], lhsT=ones_sb[:, :],
                            rhs=decb_sb[:, 512 * vt: 512 * vt + 512],
                            start=False, stop=True,
                        )
                        ex = dsp.tile([128, 512], bf16, tag="ex")
                        sp = dsp.tile([128, 1], f32, tag="sp")
                        nc.scalar.activation(ex[:, :], lg[:, :], AF.Exp,
                                             accum_out=sp[:, :])
                        if vt == 0:
                            nc.vector.tensor_copy(sums_sb[:, tb: tb + 1],
                                                  sp[:, :])
                        else:
                            nc.vector.tensor_tensor(sums_sb[:, tb: tb + 1],
                                                    sums_sb[:, tb: tb + 1],
                                                    sp[:, :], ALU.add)
                        lb = dsp.tile([128, 512], bf16, tag="lb")
                        nc.vector.tensor_copy(lb[:, :], lg[:, :])
                        nc.sync.dma_start(out=lg_tiles[tb][vt][:, :],
                                          in_=lb[:, :])

            # ---- phase 4: AllReduce sum(exp) -----------------------------
            nc.sync.dma_start(out=ar_in[:, :], in_=sums_sb[:, :])
            nc.gpsimd.collective_compute(
                "AllReduce", ALU.add,
                replica_groups=[list(range(NCORES))],
                ins=[ar_in.opt()], outs=[ar_out.opt()],
            )
            nc.sync.dma_start(out=sums_sb[:, :], in_=ar_out[:, :])
            nc.scalar.activation(nls_sb[:, :], sums_sb[:, :], AF.Ln)
            nc.scalar.activation(nls_sb[:, :], nls_sb[:, :], AF.Copy,
                                 scale=-1.0)

            # ---- phase 5: out = logits - logS ----------------------------
            with (
                tc.tile_pool(name="fin_sb", bufs=3) as fsp,
            ):
                for tb in range(NTB):
                    for vt in range(NVT):
                        li = fsp.tile([128, 512], bf16, tag="li")
                        nc.sync.dma_start(out=li[:, :],
                                          in_=lg_tiles[tb][vt][:, :])
                        ot = fsp.tile([128, 512], f32, tag="ot")
                        nc.scalar.activation(ot[:, :], li[:, :], AF.Identity,
                                             bias=nls_sb[:, tb: tb + 1])
                        nc.sync.dma_start(
                            out=out_lp[128 * tb: 128 * tb + 128,
                                       512 * vt: 512 * vt + 512],
                            in_=ot[:, :],
                        )

    nc.compile()
    return nc


# ---------------------------------------------------------------------------
# host side
# ---------------------------------------------------------------------------

def _host_prep(x, emb_W, w_ih, w_hh, b_ih, b_hh, dec_b):
    x = np.asarray(x)
    emb_W = np.asarray(emb_W, np.float32)
    w_ih = np.asarray(w_ih, np.float32)
    w_hh = np.asarray(w_hh, np.float32)
    b_ih = np.asarray(b_ih, np.float32)
    b_hh = np.asarray(b_hh, np.float32)
    dec_b = np.asarray(dec_b, np.float32)

    Bq, T = x.shape
    assert Bq == B
    vocab, Hq = emb_W.shape
    assert Hq == H and T % 16 == 0

    vsh = -(-vocab // NCORES)            # logical shard size
    VA = -(-vsh // 512) * 512            # padded to 512
    T8 = 8 * T

    # gate permutation: torch order i,f,g,o -> kernel order i,f,o,g
    perm = np.concatenate([np.arange(0, H), np.arange(H, 2 * H),
                           np.arange(3 * H, 4 * H), np.arange(2 * H, 3 * H)])

    idx = x.T.reshape(-1).astype(np.int64)          # row tb = 8t+b
    emb = emb_W[idx]                                # [T8, H]
    embT = np.ascontiguousarray(emb.T).astype(BF16)
    wihT = np.ascontiguousarray(w_ih[perm].T).astype(BF16)
    whhT = np.ascontiguousarray(w_hh[perm].T).astype(BF16)
    wihb = (b_ih + b_hh)[perm].reshape(1, -1).astype(BF16)
    i8 = np.eye(8, dtype=BF16)
    ones1 = np.ones((1, 128), dtype=BF16)

    in_maps = []
    for c in range(NCORES):
        lo = c * vsh
        hi = min(lo + vsh, vocab)
        ew = np.zeros((H, VA), dtype=BF16)
        ew[:, : hi - lo] = emb_W[lo:hi].T.astype(BF16)
        db = np.full((1, VA), NEG_PAD, dtype=BF16)
        db[0, : hi - lo] = dec_b[lo:hi].astype(BF16)
        in_maps.append({
            "embT": embT, "wihT": wihT, "wihb": wihb, "whhT": whhT,
            "embWT": ew, "decb": db, "ident8": i8, "ones1": ones1,
        })
    meta = dict(T=T, T8=T8, VA=VA, vsh=vsh, vocab=vocab)
    return in_maps, meta


def _assemble(results, meta):
    T, T8, VA, vsh, vocab = (meta[k] for k in ("T", "T8", "VA", "vsh", "vocab"))
    parts = []
    for c in range(NCORES):
        lo = c * vsh
        hi = min(lo + vsh, vocab)
        parts.append(results[c]["out_lp"][:, : hi - lo])
    lp = np.concatenate(parts, axis=1)              # [T8(tb=8t+b), vocab]
    lp = np.ascontiguousarray(
        lp.reshape(T, B, vocab).transpose(1, 0, 2).reshape(B * T, vocab))
    h = results[0]["out_h"].astype(np.float32)[None]
    c = results[0]["out_c"].astype(np.float32)[None]
    return lp, (h, c)


def _install_ntff_hook():
    """Dev-only: shim the missing antenv.axon_hooks module so trace=True
    works under axon in this container (used by test.py, not by grading)."""
    import sys, types
    if "antenv.axon_hooks" in sys.modules:
        return
    import antenv
    mod = types.ModuleType("antenv.axon_hooks")
    state = {"hook": None}
    mod.set_axon_ntff_profile_hook = lambda h: state.__setitem__("hook", h)
    mod.get_axon_ntff_profile_hook = lambda: state["hook"]
    sys.modules["antenv.axon_hooks"] = mod
    antenv.axon_hooks = mod
    try:
        from trn_agent_boot.trn_boot import _ntff_profile_via_ctypes
        mod.set_axon_ntff_profile_hook(
            _ntff_profile_via_ctypes("/opt/axon/libaxon_pjrt.so"))
    except Exception as e:  # degrade to no tracing
        print("ntff hook install failed:", e)


def kernel(x, emb_W, w_ih, w_hh, b_ih, b_hh, dec_b, _trace=False):
    in_maps, meta = _host_prep(x, emb_W, w_ih, w_hh, b_ih, b_hh, dec_b)
    key = (meta["T"], meta["VA"])
    if key not in _BUILD_CACHE:
        _BUILD_CACHE[key] = build_module(meta["T"], meta["VA"])
    nc = _BUILD_CACHE[key]

    import concourse.bass_utils as bu
    from concourse.bass_utils import run_bass_kernel_spmd
    kw = {}
    if _trace:
        _install_ntff_hook()
        bu.upload_artifacts = lambda d: "local://" + d
        kw = dict(trace=True, trace_cores=list(range(NCORES)))
    res = run_bass_kernel_spmd(nc, in_maps, core_ids=list(range(NCORES)), **kw)
    out = _assemble(res.results, meta)
    if _trace:
        return out, res
    return out
